# revision 45
# baseline (speedup 1.0000x reference)
"""Trainium2 Bass kernel for a 7-layer ternary-weight (BitNet) 1D conv
feature extractor with exact-erf GELU after each layer.

Contract: kernel(**inputs) takes the FULL inputs from setup_inputs()
(x: [8, 160000] f32, w0..w6 / b0..b6 conv params) and returns the full
output [8, 256, 500] f32.

Strategy: data-parallel over batch — one batch element per NeuronCore,
8 cores. Weights are ternarized on host (sign in {-1,0,1} is exact in
fp16; the per-tensor absmean scale is folded into the GELU activation's
per-partition scale operand). Activations are fp16 in SBUF; every conv
is computed as K accumulating matmuls (one per tap, contraction = Cin
chunk) into fp32 PSUM, with stride-2 fp16 rhs access patterns so no
deinterleaving of intermediate activations is ever needed. Layer 0
(Cin=1, K=10, stride 5) uses a host-prepared 10-row phase buffer
xr[j, t] = xpad[5t + j] so its rhs is contiguous with contraction 10.
L0's input and output are streamed through SBUF ring buffers (with a
1-column halo on the L0 output) to fit the 208KB/partition budget;
layers 1-6 keep their outputs fully resident.
"""

import numpy as np

# (in_ch, out_ch, kernel, stride, pad) — fixed problem geometry
LAYERS = [(1, 128, 10, 5, 4), (128, 192, 3, 2, 1), (192, 192, 3, 2, 1),
          (192, 192, 3, 2, 1), (192, 256, 3, 2, 1), (256, 256, 4, 2, 1),
          (256, 256, 4, 2, 1)]
T_IN = 160000
LOUT = [32000, 16000, 8000, 4000, 2000, 1000, 500]
LIN = [T_IN] + LOUT[:-1]
N_CORES = 8
NT = 512        # matmul free-dim tile (one fp32 PSUM bank)
A0C = 8192      # L0-output chunk (ring buffered), multiple of 2*NT
XTC = 4096      # L0-input chunk (ring buffered), multiple of NT


def _chunks(c):
    return [(0, min(c, 128))] + ([(128, c - 128)] if c > 128 else [])


def _wgroups(i):
    """Contraction groups of layer i (i>=1): matmul accumulation chunks.
    ("std", tile_idx, csz, tap): rows = channels [128*ti, 128*ti+csz) at tap.
    ("comb", tile_idx, tap0): 128 rows = 64 leftover channels at tap0 on
    rows 0:64 plus the SAME channels at tap0+1 on rows 64:128 — served by
    the shifted partition-duplicate in the input buffer's rows 64:128."""
    cin, cout, k, s, p = LAYERS[i]
    if cin <= 128:
        return [("std", 0, cin, kk) for kk in range(k)]
    if cin == 192:
        return ([("std", 0, 128, kk) for kk in range(3)]
                + [("comb", 1, 0), ("std", 1, 64, 2)])
    return [("std", ti, 128, kk) for ti in range(2) for kk in range(k)]


def _layout():
    """Column layout of the packed weight/bias tensors."""
    wcols = {}
    tot = 0
    for i, (cin, cout, k, s, p) in enumerate(LAYERS):
        ngroups = 1 if i == 0 else len(_wgroups(i))
        wcols[i] = tot
        tot += ngroups * cout if i != 0 else cout
    nb = 0
    bcols = {}
    for i, (cin, cout, k, s, p) in enumerate(LAYERS):
        for mi, _ in enumerate(_chunks(cout)):
            bcols[(i, mi)] = nb
            nb += 2  # bias col + scale col
    return wcols, tot, bcols, nb


def _pack_host(ws, bs):
    """Ternarize weights; pack signs (fp16) and bias+scale (fp32)."""
    wcols, tot, bcols, nb = _layout()
    wpk = np.zeros((128, tot), np.float16)
    bpk = np.zeros((128, nb), np.float32)
    for i, (cin, cout, k, s, p) in enumerate(LAYERS):
        w = np.asarray(ws[i], np.float32)
        scale = max(float(np.mean(np.abs(w))), 1e-5)
        sign = np.clip(np.round(w / scale), -1.0, 1.0)  # [cout, cin, k]
        base = wcols[i]
        if i == 0:
            wpk[0:k, base:base + cout] = sign[:, 0, :].T.astype(np.float16)
        else:
            for gi, g in enumerate(_wgroups(i)):
                if g[0] == "std":
                    _, ti, csz, kk = g
                    blk = sign[:, 128 * ti:128 * ti + csz, kk].T
                    if ti == 1 and csz == 64:
                        # B-k2 row-tiling pair: duplicate weights at rows
                        # 64:128 so odd slices can run in array rows 64-127
                        blk = np.concatenate([blk, blk], axis=0)
                else:
                    _, ti, k0 = g
                    blk = np.concatenate(
                        [sign[:, 128:192, k0].T, sign[:, 128:192, k0 + 1].T],
                        axis=0)
                wpk[0:blk.shape[0], base + gi * cout:base + (gi + 1) * cout] \
                    = blk.astype(np.float16)
        b = np.asarray(bs[i], np.float32)
        for mi, (m0, msz) in enumerate(_chunks(cout)):
            c = bcols[(i, mi)]
            bpk[0:msz, c] = b[m0:m0 + msz]
            bpk[0:msz, c + 1] = scale
    return wpk, bpk


def _prep_x(xb):
    """Per-core L0 input: xr[j, t] = xpad[5t + j], xpad = 4-zero-padded x."""
    xpad = np.zeros(T_IN + 16, np.float16)
    xpad[4:4 + T_IN] = xb.astype(np.float16)
    L = LOUT[0]
    xr = np.empty((10, L), np.float16)
    for j in range(10):
        xr[j, :] = xpad[j:j + 5 * L:5]
    return xr


_CACHE = {}


def _build():
    """Build + compile the Bass program (weight-data-independent)."""
    if "nc" in _CACHE:
        return _CACHE["nc"]
    from concourse import bacc
    import concourse.mybir as mybir
    import concourse.tile as tile

    F16 = mybir.dt.float16
    F32 = mybir.dt.float32
    GELU = mybir.ActivationFunctionType.Gelu
    wcols, tot, bcols, nb = _layout()

    nc = bacc.Bacc("TRN2")
    xr_d = nc.dram_tensor("xr", [10, LOUT[0]], F16, kind="ExternalInput")
    wp_d = nc.dram_tensor("wp", [128, tot], F16, kind="ExternalInput")
    bp_d = nc.dram_tensor("bp", [128, nb], F32, kind="ExternalInput")
    y_d = nc.dram_tensor("y", [256, 500], F32, kind="ExternalOutput")

    with tile.TileContext(nc) as tc:
        pools = []

        def mkpool(name, bufs=1, space="SBUF"):
            p = tc.alloc_tile_pool(name=name, bufs=bufs, space=space)
            pools.append(p)
            return p

        wpool = mkpool("wpool")
        wt = wpool.tile([128, tot], F16, name="wt")
        bt = wpool.tile([128, nb], F32, name="bt")

        opool = mkpool("opool")
        stage = opool.tile([128, 1000], F32, name="stage")
        scratch = opool.tile([128, 512], F16, name="scratch")
        xpool = mkpool("xpool", bufs=3)
        a0pool = mkpool("a0pool", bufs=2)

        # fully-resident output buffers for layers 1..5 (+ pad columns).
        # Always 128 partitions: for Cout=192 layers the second tile's rows
        # 64:128 hold a 1-col-left-shifted duplicate of rows 0:64 (written
        # by a DMA), serving the "comb" two-taps-in-one-matmul groups.
        act_tiles = {}
        for i in range(1, 6):
            cout = LAYERS[i][1]
            lout = LOUT[i]
            pool = mkpool(f"apool{i}")
            tiles = []
            for mi, (m0, msz) in enumerate(_chunks(cout)):
                t = pool.tile([128, lout + 4], F16, name=f"a{i}_{mi}")
                nc.vector.memset(t[:, 0:1], 0.0)
                nc.vector.memset(t[:, lout + 1:lout + 3], 0.0)
                tiles.append(t)
            act_tiles[i] = tiles

        # shifted-duplicate progress (in output-buffer columns) per layer
        dup_prog = {1: 0, 2: 0, 3: 0}

        def dup_advance(i, upto):
            """Extend rows 64:128 of layer i's second tile: out col b gets
            in col b+1 (i.e. dup[p, b] = orig[p, b+1], a 1-col left shift)."""
            p = dup_prog[i]
            if upto <= p:
                return
            b2 = act_tiles[i][1]
            nc.sync.dma_start(out=b2[64:128, p:upto],
                              in_=b2[0:64, p + 1:upto + 1])
            dup_prog[i] = upto

        # PSUM pool for the L0/L1 phase: 4 rotating slots x 2 banks.
        # Junk warm-up matmuls borrow ring slots instead of a fixed bank.
        pspool = tc.alloc_tile_pool(name="pspoolA", bufs=4, space="PSUM")
        SNT = 2 * NT  # supertile width for L0/L1

        # PE warm-up: junk matmuls on a zeroed scratch tile so the HAM
        # clock-gate reaches 8/8 before (and while) the first input DMAs
        # land; otherwise the first ~25us of real matmuls run at 1.2 GHz.
        nc.vector.memset(scratch[:, :], 0.0)

        def junk_mms(n):
            jp = pspool.tile([128, NT], F32, name="ps", tag="ps")
            for _ in range(n):
                nc.tensor.matmul(jp[:, :], scratch[:, 0:128],
                                 scratch[:, :], start=True, stop=True)

        junk_mms(26)

        cur_ps = [pspool]

        def emit_group(i, t0, nst, mi, rhs_of, dst_of):
            """One (supertile, cout-chunk) unit of layer i: per 512-col
            slice an accumulating matmul group; one batched GELU over all
            banks. rhs_of(g, tt, n) -> rhs AP for group descriptor g."""
            cin, cout, k, s, p = LAYERS[i]
            groups = [("l0",)] if i == 0 else _wgroups(i)
            n_acc = len(groups)
            m0, msz = _chunks(cout)[mi]
            ps = cur_ps[0].tile([msz, nst], F32, name="ps", tag="ps")
            # weight-outer order: consecutive matmuls share the same
            # stationary operand across the PSUM bank slices
            for a, g in enumerate(groups):
                csz = 10 if i == 0 else (128 if g[0] == "comb" else g[2])
                wb = wcols[i] + a * cout + m0
                lhsT = wt[0:csz, wb:wb + msz]
                # B-k2 (64-row leftover tap): odd slices use array rows
                # 64-127 (weights + shifted-dup data live there too) so
                # adjacent slice pairs execute concurrently via row tiling
                pairable = (i >= 2 and g[0] == "std" and g[1] == 1
                            and g[2] == 64)
                lhsT_hi = wt[64:128, wb:wb + msz] if pairable else None
                for ji, j0 in enumerate(range(0, nst, NT)):
                    n = min(NT, nst - j0)
                    if pairable and ji % 2 == 1:
                        nc.tensor.matmul(
                            ps[:, j0:j0 + n], lhsT_hi,
                            rhs_of(g, t0 + j0, n, hi=True),
                            start=(a == 0), stop=(a == n_acc - 1))
                    else:
                        nc.tensor.matmul(
                            ps[:, j0:j0 + n], lhsT,
                            rhs_of(g, t0 + j0, n),
                            start=(a == 0), stop=(a == n_acc - 1))
            bc = bcols[(i, mi)]
            nc.scalar.activation(dst_of(mi, msz), ps[0:msz, 0:nst], GELU,
                                 bias=bt[0:msz, bc:bc + 1],
                                 scale=bt[0:msz, bc + 1:bc + 2])

        def emit_supertile(i, t0, nst, rhs_of, dst_of):
            for mi in range(len(_chunks(LAYERS[i][1]))):
                emit_group(i, t0, nst, mi, rhs_of, dst_of)

        # ---- layers 0+1 interleaved over A0C-sized chunks of L0 output ----
        # a0 chunk tile: col j holds L0-output u = cbase-1+j (col 0 = halo)
        n_ch = (LOUT[0] + A0C - 1) // A0C
        a0_tiles = [None] * n_ch

        def l1_units(c, cbase, csz):
            """L1 (supertile, mchunk) emitter thunks for a0 chunk c."""
            units = []
            src_of = lambda: a0_tiles[c]
            for t0 in range(cbase // 2, (cbase + csz) // 2, SNT):
                nst = min(SNT, (cbase + csz) // 2 - t0)
                for mi in range(2):
                    def u(t0=t0, nst=nst, mi=mi):
                        src = src_of()
                        emit_group(
                            1, t0, nst, mi,
                            lambda g, tt, n: src[0:128,
                                                 2 * tt + g[3] - cbase:
                                                 2 * tt + g[3] - cbase
                                                 + 2 * n:2],
                            lambda mi, msz, t0=t0, nst=nst:
                                act_tiles[1][mi][0:msz, 1 + t0:1 + t0 + nst])
                    units.append(u)
            return units

        def mk_deep_unit(i, t0, nst):
            """Emitter thunk for one layer-i (i>=2) supertile (both cout
            chunks) + the shifted-dup advance its consumers need."""
            def u():
                lout = LOUT[i]
                if i < 6:
                    def dst(mi, msz):
                        return act_tiles[i][mi][0:msz, 1 + t0:1 + t0 + nst]
                else:
                    def dst(mi, msz):
                        return stage[0:msz, 500 * mi + t0:500 * mi + t0 + nst]
                def rhs(g, tt, n, hi=False):
                    src = act_tiles[i - 1][g[1]]
                    if g[0] == "comb":
                        return src[0:128, 2 * tt:2 * tt + 2 * n:2]
                    if hi:
                        # shifted dup rows: B2[64+p, 2t+1] == B[p, 2t+2]
                        return src[64:128, 2 * tt + 1:2 * tt + 1 + 2 * n:2]
                    kk = g[3]
                    return src[0:g[2], 2 * tt + kk:2 * tt + kk + 2 * n:2]
                emit_supertile(i, t0, nst, rhs, dst)
                if i in (2, 3):
                    dup_advance(i, t0 + nst if t0 + nst < lout else lout + 1)
            return u

        wrest = [0]

        def after_first_xt():
            # Bulk weight DMA goes via SWDGE (gpsimd) so it shares SDMA
            # round-robin with — instead of queuing ahead of — the
            # latency-critical x-chunk DMAs on the HWDGE path.
            if wrest[0] == 1:
                l1end = wcols[2]
                nc.gpsimd.dma_start(out=wt[:, 128:l1end],
                                    in_=wp_d.ap()[:, 128:l1end])
                nc.gpsimd.dma_start(out=wt[:, l1end:tot],
                                    in_=wp_d.ap()[:, l1end:tot])
            wrest[0] += 1

        for c in range(n_ch):
            cbase = c * A0C
            csz = min(A0C, LOUT[0] - cbase)
            at = a0pool.tile([128, A0C + 3], F16, tag="a0", name=f"a0_{c}")
            a0_tiles[c] = at
            if c == 0:
                nc.vector.memset(at[:, 0:1], 0.0)
            else:
                # left halo: duplicate previous chunk's last output column
                nc.vector.tensor_copy(at[:, 0:1],
                                      a0_tiles[c - 1][:, A0C:A0C + 1])
            # filler units woven between L0 supertiles: L1 of the previous
            # chunk (or junk matmuls during chunk 0 to keep the PE dense
            # and the HAM clock-gate warm).
            if c > 0:
                fillers = l1_units(c - 1, (c - 1) * A0C, A0C)
            else:
                fillers = [lambda: junk_mms(3) for _ in range(8)]
            fi = 0
            # chunk 0: halve the first x DMA so compute starts sooner
            xbs = ([cbase, cbase + XTC // 2, cbase + XTC] if c == 0
                   else list(range(cbase, cbase + csz, XTC)))
            for xb in xbs:
                xn = min(XTC // 2 if c == 0 and xb < XTC else XTC,
                         LOUT[0] - xb)
                xt = xpool.tile([10, XTC], F16, tag="xt", name=f"xt_{xb}")
                nc.sync.dma_start(out=xt[:, 0:xn],
                                  in_=xr_d.ap()[:, xb:xb + xn])
                if xb == 0:
                    # L0 weights + biases right after the first x chunk;
                    # the bulk weight DMA goes later via SWDGE.
                    nc.sync.dma_start(out=wt[:, 0:128],
                                      in_=wp_d.ap()[:, 0:128])
                    nc.sync.dma_start(out=bt[:, :], in_=bp_d.ap())
                after_first_xt()
                for t0 in range(xb, xb + xn, SNT):
                    nst = min(SNT, xb + xn - t0)
                    emit_supertile(
                        0, t0, nst,
                        lambda g, tt, n, xb=xb: xt[0:10, tt - xb:
                                                   tt - xb + n],
                        lambda mi, msz, t0=t0, nst=nst, cbase=cbase:
                            at[0:msz, t0 - cbase + 1:t0 - cbase + 1 + nst])
                    if fi < len(fillers):
                        fillers[fi]()
                        fi += 1
            while fi < len(fillers):
                fillers[fi]()
                fi += 1
            if c > 0:
                dup_advance(1, c * A0C // 2)
        for u in l1_units(n_ch - 1, (n_ch - 1) * A0C, csz):
            u()
        dup_advance(1, LOUT[1] + 1)

        # ---- layers 2..6 on fully-resident buffers ----
        # switch PSUM to 2 slots x 4 banks: groups here are 24 matmuls,
        # so wider GELU batches win and slot coupling is not a concern.
        pspool.release()
        pspoolB = tc.alloc_tile_pool(name="pspoolB", bufs=2, space="PSUM")
        cur_ps[0] = pspoolB
        SNT2 = 4 * NT
        for i in range(2, 7):
            lout = LOUT[i]
            for t0 in range(0, lout, SNT2):
                nst = min(SNT2, lout - t0)
                mk_deep_unit(i, t0, nst)()

        nc.sync.dma_start(out=y_d.ap()[0:128, :], in_=stage[:, 0:500])
        nc.sync.dma_start(out=y_d.ap()[128:256, :], in_=stage[:, 500:1000])
        pspoolB.release()
        for p in reversed(pools):
            p.release()

    nc.compile()
    _CACHE["nc"] = nc
    return nc


def kernel(x, w0, b0, w1, b1, w2, b2, w3, b3, w4, b4, w5, b5, w6, b6):
    import os
    from concourse.bass_utils import run_bass_kernel_spmd

    ws = [w0, w1, w2, w3, w4, w5, w6]
    bs = [b0, b1, b2, b3, b4, b5, b6]
    wpk, bpk = _pack_host(ws, bs)
    x = np.asarray(x, np.float32)
    in_maps = [{"xr": _prep_x(x[b]), "wp": wpk, "bp": bpk}
               for b in range(N_CORES)]
    nc = _build()
    trace = bool(os.environ.get("BITCONV_TRACE"))
    res = run_bass_kernel_spmd(nc, in_maps, core_ids=list(range(N_CORES)),
                               trace=trace)
    if trace:
        print(f"HW exec time: {res.exec_time_ns} ns")
        _CACHE["last_results"] = res
    return np.stack([res.results[b]["y"] for b in range(N_CORES)], axis=0)
